# revision 18
# baseline (speedup 1.0000x reference)
"""GQA attention kernel for 8 TRN2 NeuronCores.

Sharding (hardcoded): 8 cores = batch(2) x kv-group(4).
Core i handles batch b=i//4, group g=i%4:
  x  = hidden_states[b]            [2048, 2048]
  wq = Wq[:, g*512:(g+1)*512]      [2048, 512]   (8 q heads)
  wk = Wk[:, g*128:(g+1)*128]      [2048, 128]   (2 kv heads)
  wv = Wv[:, g*128:(g+1)*128]      [2048, 128]
  wo = Wo[g*512:(g+1)*512, :]      [512, 2048]   (rows permuted on host)
Each core returns a partial output [2048, 2048]; host sums the 4 group
partials per batch.

Per-core pipeline (all matmuls bf16 -> f32 PSUM):
  A) cast X f32->bf16 by 128-row chunks (SWDGE DMA) into DRAM staging,
     HW-transpose-DMA each [128,128] block into X^T bf16 in SBUF.
  B) QKV projections in [tok, dim] layout (lhsT = X^T blocks), RoPE on
     the free dim (tables precomputed for all positions), PE-transpose
     Q/K to Q^T/K^T layout; V kept [tok, d] with a ones column appended
     per kv head (for softmax row sums).
  C) per (q-chunk, head-pair): scores for the kv0 head (PE rows 0-63)
     and kv1 head (rows 64-127) issued back-to-back -> the row-tiled
     matmuls run concurrently at full array width (keeps the PE HAM
     activity monitor warm / 2.4 GHz).  exp runs as one [128,1024] ACT
     instruction over both PSUM banks; PV lags one slot so PE never
     waits on ACT.  Wo matmuls for the previous q-chunk are interleaved
     one per slot to fill PE slack.
  D) leftover Wo work drains after the attention loops.
"""

import math
import numpy as np

S = 2048
HID = 2048
NT = 16          # token tiles of 128
NR = 16          # hid tiles of 128
QD = 512         # q dims per core (8 heads x 64)
KD = 128         # kv dims per core (2 heads x 64)
D = 64
NQH = 8          # q heads per core
PI = math.pi

_CACHE = {}


def _build():
    import concourse.bass as bass
    import concourse.mybir as mybir
    from concourse import bacc
    from concourse.tile import TileContext
    from concourse.masks import make_identity

    f32 = mybir.dt.float32
    bf16 = mybir.dt.bfloat16
    i32 = mybir.dt.int32
    AF = mybir.ActivationFunctionType
    OP = mybir.AluOpType

    nc = bacc.Bacc("TRN2", target_bir_lowering=False, debug=False)
    x = nc.dram_tensor("x", [S, HID], f32, kind="ExternalInput").ap()
    wq = nc.dram_tensor("wq", [HID, QD], f32, kind="ExternalInput").ap()
    wk = nc.dram_tensor("wk", [HID, KD], f32, kind="ExternalInput").ap()
    wv = nc.dram_tensor("wv", [HID, KD], f32, kind="ExternalInput").ap()
    wo = nc.dram_tensor("wo", [QD, HID], f32, kind="ExternalInput").ap()
    out = nc.dram_tensor("out", [S, HID], f32, kind="ExternalOutput").ap()
    import os
    dbg = os.environ.get("KDEBUG") == "1"
    if dbg:
        d_xt = nc.dram_tensor("d_xt", [128, NR, S], f32,
                              kind="ExternalOutput").ap()
        d_sin = nc.dram_tensor("d_sin", [128, NT, 64], f32,
                               kind="ExternalOutput").ap()
        d_cos = nc.dram_tensor("d_cos", [128, NT, 64], f32,
                               kind="ExternalOutput").ap()
        d_qt = nc.dram_tensor("d_qt", [128, 4, S], f32,
                              kind="ExternalOutput").ap()
        d_kt = nc.dram_tensor("d_kt", [128, S], f32,
                              kind="ExternalOutput").ap()
        d_v = nc.dram_tensor("d_v", [128, NT, 2, 65], f32,
                             kind="ExternalOutput").ap()
        d_at = nc.dram_tensor("d_at", [128, 4, S], f32,
                              kind="ExternalOutput").ap()
        d_sc = nc.dram_tensor("d_sc", [128, 2, 512], f32,
                              kind="ExternalOutput").ap()
        d_p = nc.dram_tensor("d_p", [128, 2, 512], f32,
                             kind="ExternalOutput").ap()
        d_o = nc.dram_tensor("d_o", [128, 2, 512], f32,
                             kind="ExternalOutput").ap()

    with TileContext(nc) as tc:
        with (
            tc.tile_pool(name="dram", bufs=1, space="DRAM") as dram,
            tc.tile_pool(name="const", bufs=1) as const,
            tc.tile_pool(name="wts", bufs=1) as wts,
            tc.tile_pool(name="xt", bufs=1) as xtp,
            tc.tile_pool(name="stage", bufs=2) as stage,
            tc.tile_pool(name="tmps", bufs=3) as tmps,
            tc.tile_pool(name="pbf", bufs=3) as pbf,
            tc.tile_pool(name="rbp", bufs=2) as rbp,
            tc.tile_pool(name="outp", bufs=3) as outp,
        ):
            # ---------------- weights -> SBUF bf16 ----------------
            wq_sb = wts.tile([128, NR, QD], bf16, tag="wq")
            wkv_sb = wts.tile([128, NR, 2 * KD], bf16, tag="wkv")
            wo_sb = wts.tile([128, 4, HID], bf16, tag="wo")
            nc.gpsimd.dma_start(
                out=wq_sb[:], in_=wq.rearrange("(r p) q -> p r q", p=128))
            nc.gpsimd.dma_start(
                out=wkv_sb[:, :, 0:KD],
                in_=wk.rearrange("(r p) q -> p r q", p=128))
            nc.gpsimd.dma_start(
                out=wkv_sb[:, :, KD:2 * KD],
                in_=wv.rearrange("(r p) q -> p r q", p=128))
            nc.gpsimd.dma_start(
                out=wo_sb[:], in_=wo.rearrange("(d p) n -> p d n", p=128))

            ident = const.tile([128, 128], bf16, tag="ident")
            make_identity(nc, ident[:])

            # ---------------- Phase A: X^T bf16 in SBUF ----------------
            # per 128-token chunk: cast-DMA to DRAM bf16 staging, then 16
            # HW transpose-DMAs into the X^T column slice.
            xT = []
            for r in range(NR):
                xt_r = xtp.tile([128, S], bf16, tag=f"xt{r}")
                xT.append(xt_r)
            for t in range(NT):
                xbc = dram.tile([128, HID], bf16, tag=f"xb{t}")
                nc.gpsimd.dma_start(
                    out=xbc[:], in_=x[t * 128:(t + 1) * 128, :])
                for r in range(NR):
                    nc.sync.dma_start(
                        out=xT[r][:, t * 128:(t + 1) * 128],
                        in_=xbc[:, r * 128:(r + 1) * 128], transpose=True)

            # ---------------- RoPE tables (all positions, batched) ------
            # inv_freq[i] = 10000^(-i/32), i = d mod 32, broadcast to rows
            it32 = const.tile([1, 32], i32, tag="it32")
            nc.gpsimd.iota(it32[:], pattern=[[1, 32]], base=0,
                           channel_multiplier=0)
            invf_row = const.tile([1, 64], f32, tag="invf_row")
            nc.vector.tensor_copy(invf_row[:, 0:32], it32[:])
            nc.vector.tensor_copy(invf_row[:, 32:64], it32[:])
            nc.scalar.activation(invf_row[:], invf_row[:], AF.Exp,
                                 scale=-math.log(10000.0) / 32.0)
            invf = const.tile([128, 64], f32, tag="invf")
            nc.gpsimd.partition_broadcast(invf[:], invf_row[:])
            pos_i = const.tile([128, NT], i32, tag="pos_i")
            nc.gpsimd.iota(pos_i[:], pattern=[[128, NT]], base=0,
                           channel_multiplier=1)
            pos_f = const.tile([128, NT], f32, tag="pos_f")
            nc.vector.tensor_copy(pos_f[:], pos_i[:])

            # frq_all[p, t, d] = pos[p, t] * invf[d]
            frq_all = const.tile([128, NT, 64], f32, tag="frq_all")
            nc.vector.tensor_tensor(
                frq_all[:],
                pos_f[:, :, None].broadcast_to([128, NT, 64]),
                invf[:, None, :].broadcast_to([128, NT, 64]),
                OP.mult)
            sin_all = const.tile([128, NT, 64], f32, tag="sin_all")
            cos_all = const.tile([128, NT, 64], f32, tag="cos_all")
            # range-reduce to [-pi, pi]: red = a - 2pi*round(a/2pi)
            # (f32->i32 tensor_copy rounds to nearest on DVE)
            NH = NT // 2
            for half in range(2):
                hs = slice(half * NH, (half + 1) * NH)
                red = tmps.tile([128, NH, 64], f32, tag="red", bufs=1)
                ki = tmps.tile([128, NH, 64], i32, tag="ki", bufs=1)
                kf = tmps.tile([128, NH, 64], f32, tag="kf", bufs=1)
                for (dst, arg_off) in ((sin_all, 0.0), (cos_all, PI / 2)):
                    a = tmps.tile([128, NH, 64], f32, tag="arg", bufs=1)
                    nc.vector.tensor_scalar(
                        out=a[:], in0=frq_all[:, hs, :], scalar1=arg_off,
                        scalar2=None, op0=OP.add)
                    nc.vector.tensor_scalar(
                        out=red[:], in0=a[:], scalar1=1.0 / (2 * PI),
                        scalar2=None, op0=OP.mult)
                    nc.vector.tensor_copy(ki[:], red[:])
                    nc.vector.tensor_copy(kf[:], ki[:])
                    nc.vector.scalar_tensor_tensor(
                        out=red[:], in0=kf[:], scalar=-2 * PI, in1=a[:],
                        op0=OP.mult, op1=OP.add)
                    nc.scalar.activation(dst[:, hs, :], red[:], AF.Sin)

            # outputs of phase B
            QT = wts.tile([128, 4, S], bf16, tag="QT")    # [qdim, m, tok]
            KT = wts.tile([128, S], bf16, tag="KT")       # [kdim(2h), tok]
            V = wts.tile([128, NT, 2, 65], bf16, tag="V")  # [tok128,t,kvh,d+1]
            nc.vector.memset(V[:, :, :, 64:65], 1.0)
            attnT = wts.tile([128, 4, S], bf16, tag="attnT")

            # ---------------- Phase B: QKV + RoPE + transposes --------
            with (
                tc.tile_pool(name="psq", bufs=2, space="PSUM") as psq,
                tc.tile_pool(name="pskv", bufs=2, space="PSUM") as pskv,
                tc.tile_pool(name="pstp", bufs=2, space="PSUM") as pstp,
            ):
                for t in range(NT):
                    cos_t = cos_all[:, t, :]
                    sin_t = sin_all[:, t, :]
                    ps_q = psq.tile([128, QD], f32, tag="q")
                    ps_kv = pskv.tile([128, 2 * KD], f32, tag="kv")
                    for r in range(NR):
                        nc.tensor.matmul(
                            ps_q[:], lhsT=xT[r][:, t * 128:(t + 1) * 128],
                            rhs=wq_sb[:, r, :],
                            start=(r == 0), stop=(r == NR - 1))
                        nc.tensor.matmul(
                            ps_kv[:], lhsT=xT[r][:, t * 128:(t + 1) * 128],
                            rhs=wkv_sb[:, r, :],
                            start=(r == 0), stop=(r == NR - 1))

                    qk = stage.tile([128, QD + KD], bf16, tag="qk")
                    # ---- RoPE on q (8 heads) and k (2 heads), free-dim
                    # layout.  Q output heads are permuted: head h -> col
                    # (h%4)*128 + (h//4)*64, so that after transpose head h
                    # sits at QT tile h%4, partition half (h//4)*64 == its
                    # kv head's partition base (wo rows are permuted on the
                    # host to match).
                    for (src, n_h, off) in ((ps_q, NQH, 0), (ps_kv, 2, QD)):
                        if n_h == NQH:
                            v3 = src[:, 0:512].rearrange(
                                "p (half blk d) -> p half blk d", half=2,
                                d=64)
                            o3 = qk[:, 0:512].rearrange(
                                "p (blk half d) -> p half blk d", half=2,
                                d=64)
                            sh = [128, 2, 4, 32]
                            c1 = cos_t[:, None, None, 0:32].broadcast_to(sh)
                            s1 = sin_t[:, None, None, 0:32].broadcast_to(sh)
                            c2 = cos_t[:, None, None, 32:64].broadcast_to(sh)
                            s2 = sin_t[:, None, None, 32:64].broadcast_to(sh)
                            q1, q2 = v3[:, :, :, 0:32], v3[:, :, :, 32:64]
                            oa, ob = o3[:, :, :, 0:32], o3[:, :, :, 32:64]
                        else:
                            v3 = src[:, 0:n_h * 64].rearrange(
                                "p (h d) -> p h d", d=64)
                            o3 = qk[:, off:off + n_h * 64].rearrange(
                                "p (h d) -> p h d", d=64)
                            sh = [128, n_h, 32]
                            c1 = cos_t[:, None, 0:32].broadcast_to(sh)
                            s1 = sin_t[:, None, 0:32].broadcast_to(sh)
                            c2 = cos_t[:, None, 32:64].broadcast_to(sh)
                            s2 = sin_t[:, None, 32:64].broadcast_to(sh)
                            q1, q2 = v3[:, :, 0:32], v3[:, :, 32:64]
                            oa, ob = o3[:, :, 0:32], o3[:, :, 32:64]
                        t1 = tmps.tile(sh, f32, tag="t1")
                        t2 = tmps.tile(sh, f32, tag="t2")
                        nc.vector.tensor_tensor(t1[:], q1, c1, OP.mult)
                        nc.vector.tensor_tensor(t2[:], q2, s1, OP.mult)
                        nc.vector.tensor_tensor(oa, t1[:], t2[:], OP.subtract)
                        nc.vector.tensor_tensor(t1[:], q2, c2, OP.mult)
                        nc.vector.tensor_tensor(t2[:], q1, s2, OP.mult)
                        nc.vector.tensor_tensor(ob, t1[:], t2[:], OP.add)
                    # ---- V evacuation (+ ones col already memset)
                    nc.vector.tensor_copy(
                        V[:, t, :, 0:64],
                        ps_kv[:, KD:2 * KD].rearrange(
                            "p (h d) -> p h d", d=64))
                    # ---- transpose q/k blocks into QT/KT
                    for db in range(5):
                        tp = pstp.tile([128, 128], bf16, tag="tp")
                        nc.tensor.transpose(
                            tp[:], qk[:, db * 128:(db + 1) * 128], ident[:])
                        if db < 4:
                            dst = QT[:, db, t * 128:(t + 1) * 128]
                        else:
                            dst = KT[:, t * 128:(t + 1) * 128]
                        nc.vector.tensor_copy(dst, tp[:])

            # ---------------- Phase C: attention (+ Wo interleaved) -----
            # Wo micro-op generator: yields per-call one PE matmul (or
            # schedules the DVE copy / out DMA when a group completes).
            with (
                tc.tile_pool(name="psS", bufs=2, space="PSUM") as psS,
                tc.tile_pool(name="psO", bufs=1, space="PSUM") as psO,
                tc.tile_pool(name="psW", bufs=2, space="PSUM") as psW,
            ):
                wo_state = {"t": 0, "nch": 0, "db": 0, "w_ps": None}

                def wo_step(max_t):
                    # emit one Wo matmul if work for token tiles < max_t
                    # remains
                    st = wo_state
                    if st["t"] >= max_t:
                        return
                    if st["w_ps"] is None:
                        st["w_ps"] = psW.tile([128, 512], f32, tag="w",
                                              name="w_ps")
                    t, nch, db = st["t"], st["nch"], st["db"]
                    nc.tensor.matmul(
                        st["w_ps"][:],
                        lhsT=attnT[:, db, t * 128:(t + 1) * 128],
                        rhs=wo_sb[:, db, nch * 512:(nch + 1) * 512],
                        start=(db == 0), stop=(db == 3))
                    st["db"] += 1
                    if st["db"] == 4:
                        st["db"] = 0
                        o_c = outp.tile([128, 512], f32, tag="out",
                                        name="o_c")
                        nc.vector.tensor_copy(o_c[:], st["w_ps"][:])
                        nc.sync.dma_start(
                            out=out[t * 128:(t + 1) * 128,
                                    nch * 512:(nch + 1) * 512],
                            in_=o_c[:])
                        st["w_ps"] = None
                        st["nch"] += 1
                        if st["nch"] == 4:
                            st["nch"] = 0
                            st["t"] += 1

                for qc in range(4):
                    for m in range(4):
                        # heads: kv0 head m (rows 0:64), kv1 head m
                        # (rows 64:128)
                        o_A = psO.tile([65, 512], f32, tag="oA")
                        o_B = psO.tile([65, 512], f32, tag="oB")
                        prev_p = None
                        for kt in range(NT):
                            scA = psS.tile([128, 512], f32, tag="scA")
                            scB = psS.tile([128, 512], f32, tag="scB")
                            nc.tensor.matmul(
                                scA[:],
                                lhsT=KT[0:64, kt * 128:(kt + 1) * 128],
                                rhs=QT[0:64, m, qc * 512:(qc + 1) * 512],
                                start=True, stop=True)
                            nc.tensor.matmul(
                                scB[:],
                                lhsT=KT[64:128, kt * 128:(kt + 1) * 128],
                                rhs=QT[64:128, m, qc * 512:(qc + 1) * 512],
                                start=True, stop=True)
                            pA = pbf.tile([128, 512], bf16, tag="pA")
                            pB = pbf.tile([128, 512], bf16, tag="pB")
                            nc.scalar.activation(pA[:], scA[:], AF.Exp,
                                                 scale=0.125)
                            nc.scalar.activation(pB[:], scB[:], AF.Exp,
                                                 scale=0.125)
                            if dbg and qc == 0 and m == 0 and kt == 0:
                                dsc = stage.tile([128, 2, 512], f32,
                                                 tag="dsc", bufs=1)
                                nc.vector.tensor_copy(dsc[:, 0, :], scA[:])
                                nc.vector.tensor_copy(dsc[:, 1, :], scB[:])
                                nc.gpsimd.dma_start(out=d_sc, in_=dsc[:])
                                dp = stage.tile([128, 2, 512], f32,
                                                tag="dp", bufs=1)
                                nc.vector.tensor_copy(dp[:, 0, :], pA[:])
                                nc.vector.tensor_copy(dp[:, 1, :], pB[:])
                                nc.gpsimd.dma_start(out=d_p, in_=dp[:])
                            # PV for the previous slot's p (keeps PE off
                            # the ACT critical path)
                            if prev_p is not None:
                                ppA, ppB, pkt = prev_p
                                nc.tensor.matmul(
                                    o_A[:], lhsT=V[:, pkt, 0, :],
                                    rhs=ppA[:],
                                    start=(pkt == 0), stop=(pkt == NT - 1))
                                nc.tensor.matmul(
                                    o_B[:], lhsT=V[:, pkt, 1, :],
                                    rhs=ppB[:],
                                    start=(pkt == 0), stop=(pkt == NT - 1))
                            wo_step(qc * 4)
                            prev_p = (pA, pB, kt)
                        ppA, ppB, pkt = prev_p
                        nc.tensor.matmul(
                            o_A[:], lhsT=V[:, pkt, 0, :], rhs=ppA[:],
                            start=(pkt == 0), stop=(pkt == NT - 1))
                        nc.tensor.matmul(
                            o_B[:], lhsT=V[:, pkt, 1, :], rhs=ppB[:],
                            start=(pkt == 0), stop=(pkt == NT - 1))
                        if dbg and qc == 0 and m == 0:
                            do_ = stage.tile([128, 2, 512], f32,
                                             tag="do", bufs=1)
                            nc.vector.tensor_copy(do_[0:65, 0, :], o_A[:])
                            nc.vector.tensor_copy(do_[0:65, 1, :], o_B[:])
                            nc.gpsimd.dma_start(out=d_o, in_=do_[:])
                        # normalize: row 64 carries the softmax denominator
                        for (o_ps, qr) in ((o_A, 0), (o_B, 64)):
                            rsum = rbp.tile([1, 512], f32, tag="rsum",
                                            bufs=1)
                            nc.vector.tensor_copy(rsum[:], o_ps[64:65, :])
                            recip = rbp.tile([1, 512], f32, tag="recip",
                                             bufs=1)
                            nc.vector.reciprocal_approx_fast(recip[:],
                                                             rsum[:])
                            rb = rbp.tile([64, 512], f32, tag="rb", bufs=1)
                            nc.gpsimd.partition_broadcast(rb[:], recip[:])
                            nc.vector.tensor_tensor(
                                attnT[qr:qr + 64, m,
                                      qc * 512:(qc + 1) * 512],
                                o_ps[0:64, :], rb[:], OP.mult)

                # ---------------- Phase D: drain remaining Wo ----------
                while wo_state["t"] < NT:
                    wo_step(NT)

                if dbg:
                    for r in range(NR):
                        nc.gpsimd.dma_start(out=d_xt[:, r, :], in_=xT[r][:])
                    for (dt_, st_) in ((d_sin, sin_all), (d_cos, cos_all),
                                       (d_qt, QT), (d_kt, KT), (d_v, V),
                                       (d_at, attnT)):
                        nc.gpsimd.dma_start(out=dt_, in_=st_[:])

    nc.compile()
    return nc


def _get_nc():
    if "nc" not in _CACHE:
        _CACHE["nc"] = _build()
    return _CACHE["nc"]


def _shard(inputs):
    hs = np.ascontiguousarray(np.asarray(inputs["hidden_states"], np.float32))
    Wq = np.asarray(inputs["Wq"], np.float32)
    Wk = np.asarray(inputs["Wk"], np.float32)
    Wv = np.asarray(inputs["Wv"], np.float32)
    Wo = np.asarray(inputs["Wo"], np.float32)
    in_maps = []
    for i in range(8):
        b, g = divmod(i, 4)
        in_maps.append({
            "x": hs[b],
            "wq": np.ascontiguousarray(Wq[:, g * 512:(g + 1) * 512]),
            "wk": np.ascontiguousarray(Wk[:, g * 128:(g + 1) * 128]),
            "wv": np.ascontiguousarray(Wv[:, g * 128:(g + 1) * 128]),
            "wo": np.ascontiguousarray(
                Wo[g * 512:(g + 1) * 512, :].reshape(8, 64, HID)[
                    [0, 4, 1, 5, 2, 6, 3, 7]].reshape(512, HID)),
        })
    return in_maps


def run(inputs, trace=False, tmpdir=None):
    """Run on 8 cores; returns (output [2,2048,2048] f32, exec_time_ns)."""
    from concourse.bass_utils import run_bass_kernel_spmd

    nc = _get_nc()
    in_maps = _shard(inputs)
    kwargs = {}
    if trace:
        import sys, types
        from trn_agent_boot.trn_boot import _ntff_profile_via_ctypes
        if "antenv.axon_hooks" not in sys.modules:
            mod = types.ModuleType("antenv.axon_hooks")
            hook = _ntff_profile_via_ctypes("/opt/axon/libaxon_pjrt.so")
            mod.get_axon_ntff_profile_hook = lambda: hook
            sys.modules["antenv.axon_hooks"] = mod
        import concourse.bass_utils as bu
        bu.upload_artifacts = lambda d: f"local://{d}"
        kwargs = {"trace": True, "tmpdir": tmpdir}
    res = run_bass_kernel_spmd(nc, in_maps, core_ids=list(range(8)), **kwargs)
    full = np.zeros((2, S, HID), np.float32)
    for i in range(8):
        b = i // 4
        full[b] += res.results[i]["out"]
    return full, res.exec_time_ns


def kernel(**inputs):
    out, _ = run(inputs)
    return out


# revision 24
# speedup vs baseline: 1.4720x; 1.4720x over previous
"""GQA attention kernel for 8 TRN2 NeuronCores.

Sharding (hardcoded): 8 cores = batch(2) x kv-group(4).
Core i handles batch b=i//4, group g=i%4:
  x  = hidden_states[b]            [2048, 2048]
  wq = Wq[:, g*512:(g+1)*512]      [2048, 512]   (8 q heads)
  wk = Wk[:, g*128:(g+1)*128]      [2048, 128]   (2 kv heads)
  wv = Wv[:, g*128:(g+1)*128]      [2048, 128]
  wo = Wo[g*512:(g+1)*512, :]      [512, 2048]   (rows permuted on host)
Each core returns a partial output [2048, 2048]; host sums the 4 group
partials per batch.

Per-core pipeline (all matmuls bf16 -> f32 PSUM):
  A) cast X f32->bf16 by 128-row chunks (SWDGE DMA) into DRAM staging,
     HW-transpose-DMA each [128,128] block into X^T bf16 in SBUF.
  B) QKV projections in [tok, dim] layout (lhsT = X^T blocks), RoPE on
     the free dim (tables precomputed for all positions), PE-transpose
     Q/K to Q^T/K^T layout; V kept [tok, d] with a ones column appended
     per kv head (for softmax row sums).
  C) per (q-chunk, head-pair): scores for the kv0 head (PE rows 0-63)
     and kv1 head (rows 64-127) issued back-to-back -> the row-tiled
     matmuls run concurrently at full array width (keeps the PE HAM
     activity monitor warm / 2.4 GHz).  exp runs as one [128,1024] ACT
     instruction over both PSUM banks; PV lags one slot so PE never
     waits on ACT.  Wo matmuls for the previous q-chunk are interleaved
     one per slot to fill PE slack.
  D) leftover Wo work drains after the attention loops.
"""

import math
import numpy as np

S = 2048
HID = 2048
NT = 16          # token tiles of 128
NR = 16          # hid tiles of 128
QD = 512         # q dims per core (8 heads x 64)
KD = 128         # kv dims per core (2 heads x 64)
D = 64
NQH = 8          # q heads per core
PI = math.pi

_CACHE = {}


def _build():
    import concourse.bass as bass
    import concourse.mybir as mybir
    from concourse import bacc
    from concourse.tile import TileContext
    from concourse.masks import make_identity

    f32 = mybir.dt.float32
    bf16 = mybir.dt.bfloat16
    i32 = mybir.dt.int32
    AF = mybir.ActivationFunctionType
    OP = mybir.AluOpType

    nc = bacc.Bacc("TRN2", target_bir_lowering=False, debug=False)
    x = nc.dram_tensor("x", [S, HID], f32, kind="ExternalInput").ap()
    wq = nc.dram_tensor("wq", [HID, QD], f32, kind="ExternalInput").ap()
    wk = nc.dram_tensor("wk", [HID, KD], f32, kind="ExternalInput").ap()
    wv = nc.dram_tensor("wv", [HID, KD], f32, kind="ExternalInput").ap()
    wo = nc.dram_tensor("wo", [QD, HID], f32, kind="ExternalInput").ap()
    out = nc.dram_tensor("out", [S, HID], f32, kind="ExternalOutput").ap()
    import os
    dbg = os.environ.get("KDEBUG") == "1"
    if dbg:
        d_xt = nc.dram_tensor("d_xt", [128, NR, S], f32,
                              kind="ExternalOutput").ap()
        d_sin = nc.dram_tensor("d_sin", [128, NT, 64], f32,
                               kind="ExternalOutput").ap()
        d_cos = nc.dram_tensor("d_cos", [128, NT, 64], f32,
                               kind="ExternalOutput").ap()
        d_qt = nc.dram_tensor("d_qt", [128, 4, S], f32,
                              kind="ExternalOutput").ap()
        d_kt = nc.dram_tensor("d_kt", [128, S], f32,
                              kind="ExternalOutput").ap()
        d_v = nc.dram_tensor("d_v", [128, NT, 2, 65], f32,
                             kind="ExternalOutput").ap()
        d_at = nc.dram_tensor("d_at", [128, 4, S], f32,
                              kind="ExternalOutput").ap()
        d_sc = nc.dram_tensor("d_sc", [128, 2, 512], f32,
                              kind="ExternalOutput").ap()
        d_p = nc.dram_tensor("d_p", [128, 2, 512], f32,
                             kind="ExternalOutput").ap()
        d_o = nc.dram_tensor("d_o", [128, 2, 512], f32,
                             kind="ExternalOutput").ap()

    with TileContext(nc) as tc:
        with (
            tc.tile_pool(name="dram", bufs=1, space="DRAM") as dram,
            tc.tile_pool(name="const", bufs=1) as const,
            tc.tile_pool(name="wts", bufs=1) as wts,
            tc.tile_pool(name="xt", bufs=1) as xtp,
            tc.tile_pool(name="stage", bufs=2) as stage,
            tc.tile_pool(name="tmps", bufs=3) as tmps,
            tc.tile_pool(name="pbf", bufs=3) as pbf,
            tc.tile_pool(name="rbp", bufs=2) as rbp,
            tc.tile_pool(name="outp", bufs=3) as outp,
        ):
            # ---------------- weights -> SBUF bf16 ----------------
            wq_sb = wts.tile([128, NR, QD], bf16, tag="wq")
            wkv_sb = wts.tile([128, NR, 2 * KD], bf16, tag="wkv")
            wo_sb = wts.tile([128, 4, HID], bf16, tag="wo")
            nc.gpsimd.dma_start(
                out=wq_sb[:], in_=wq.rearrange("(r p) q -> p r q", p=128))
            nc.gpsimd.dma_start(
                out=wkv_sb[:, :, 0:KD],
                in_=wk.rearrange("(r p) q -> p r q", p=128))
            nc.gpsimd.dma_start(
                out=wkv_sb[:, :, KD:2 * KD],
                in_=wv.rearrange("(r p) q -> p r q", p=128))
            nc.gpsimd.dma_start(
                out=wo_sb[:], in_=wo.rearrange("(d p) n -> p d n", p=128))

            ident = const.tile([128, 128], bf16, tag="ident")
            make_identity(nc, ident[:])

            # ---------------- Phase A: X^T bf16 in SBUF ----------------
            # per 128-token chunk: cast-DMA to DRAM bf16 staging, then ONE
            # HW transpose-DMA into xTall[:, :, tcols] (the 3D out AP's
            # middle dim extends the partition dim: logical transpose row
            # h = hid lands at partition h%128, r-slot h//128, i.e.
            # xTall[p, r, tok] = x[tok, r*128+p]).
            xTall = xtp.tile([128, NR, S], bf16, tag="xtall")
            xT = [xTall[:, r, :] for r in range(NR)]
            for t in range(NT):
                xbc = dram.tile([128, HID], bf16, tag=f"xb{t}")
                nc.gpsimd.dma_start(
                    out=xbc[:], in_=x[t * 128:(t + 1) * 128, :])
                nc.sync.dma_start(
                    out=xTall[:, :, t * 128:(t + 1) * 128],
                    in_=xbc[:], transpose=True)

            # ---------------- RoPE tables (all positions, batched) ------
            # inv_freq[i] = 10000^(-i/32), i = d mod 32, broadcast to rows
            it32 = const.tile([1, 32], i32, tag="it32")
            nc.gpsimd.iota(it32[:], pattern=[[1, 32]], base=0,
                           channel_multiplier=0)
            invf_row = const.tile([1, 64], f32, tag="invf_row")
            nc.vector.tensor_copy(invf_row[:, 0:32], it32[:])
            nc.vector.tensor_copy(invf_row[:, 32:64], it32[:])
            nc.scalar.activation(invf_row[:], invf_row[:], AF.Exp,
                                 scale=-math.log(10000.0) / 32.0)
            invf = const.tile([128, 64], f32, tag="invf")
            nc.gpsimd.partition_broadcast(invf[:], invf_row[:])
            pos_i = const.tile([128, NT], i32, tag="pos_i")
            nc.gpsimd.iota(pos_i[:], pattern=[[128, NT]], base=0,
                           channel_multiplier=1)
            pos_f = const.tile([128, NT], f32, tag="pos_f")
            nc.vector.tensor_copy(pos_f[:], pos_i[:])

            # frq_all[p, t, d] = pos[p, t] * invf[d]
            frq_all = const.tile([128, NT, 64], f32, tag="frq_all")
            nc.vector.tensor_tensor(
                frq_all[:],
                pos_f[:, :, None].broadcast_to([128, NT, 64]),
                invf[:, None, :].broadcast_to([128, NT, 64]),
                OP.mult)
            sin_all = const.tile([128, NT, 64], f32, tag="sin_all")
            cos_all = const.tile([128, NT, 64], f32, tag="cos_all")
            # range-reduce to [-pi, pi]: red = a - 2pi*round(a/2pi)
            # (f32->i32 tensor_copy rounds to nearest on DVE)
            NH = NT // 2
            for half in range(2):
                hs = slice(half * NH, (half + 1) * NH)
                red = tmps.tile([128, NH, 64], f32, tag="red", bufs=1)
                ki = tmps.tile([128, NH, 64], i32, tag="ki", bufs=1)
                kf = tmps.tile([128, NH, 64], f32, tag="kf", bufs=1)
                for (dst, arg_off) in ((sin_all, 0.0), (cos_all, PI / 2)):
                    a = tmps.tile([128, NH, 64], f32, tag="arg", bufs=1)
                    nc.vector.tensor_scalar(
                        out=a[:], in0=frq_all[:, hs, :], scalar1=arg_off,
                        scalar2=None, op0=OP.add)
                    nc.vector.tensor_scalar(
                        out=red[:], in0=a[:], scalar1=1.0 / (2 * PI),
                        scalar2=None, op0=OP.mult)
                    nc.vector.tensor_copy(ki[:], red[:])
                    nc.vector.tensor_copy(kf[:], ki[:])
                    nc.vector.scalar_tensor_tensor(
                        out=red[:], in0=kf[:], scalar=-2 * PI, in1=a[:],
                        op0=OP.mult, op1=OP.add)
                    nc.scalar.activation(dst[:, hs, :], red[:], AF.Sin)

            # outputs of phase B
            QT = wts.tile([128, 4, S], bf16, tag="QT")    # [qdim, m, tok]
            KT = wts.tile([128, S], bf16, tag="KT")       # [kdim(2h), tok]
            V = wts.tile([128, NT, 2, 65], bf16, tag="V")  # [tok128,t,kvh,d+1]
            nc.vector.memset(V[:, :, :, 64:65], 1.0)
            attnT = wts.tile([128, 4, S], bf16, tag="attnT")

            # ---------------- Phase B: QKV + RoPE + transposes --------
            with (
                tc.tile_pool(name="psq", bufs=2, space="PSUM") as psq,
                tc.tile_pool(name="pskv", bufs=2, space="PSUM") as pskv,
                tc.tile_pool(name="pstp", bufs=2, space="PSUM") as pstp,
            ):
                for t in range(NT):
                    cos_t = cos_all[:, t, :]
                    sin_t = sin_all[:, t, :]
                    ps_q = psq.tile([128, QD], f32, tag="q")
                    ps_kv = pskv.tile([128, 2 * KD], f32, tag="kv")
                    for r in range(NR):
                        nc.tensor.matmul(
                            ps_q[:], lhsT=xT[r][:, t * 128:(t + 1) * 128],
                            rhs=wq_sb[:, r, :],
                            start=(r == 0), stop=(r == NR - 1))
                        nc.tensor.matmul(
                            ps_kv[:], lhsT=xT[r][:, t * 128:(t + 1) * 128],
                            rhs=wkv_sb[:, r, :],
                            start=(r == 0), stop=(r == NR - 1))

                    qk = stage.tile([128, QD + KD], bf16, tag="qk")
                    # ---- RoPE on q (8 heads) and k (2 heads), free-dim
                    # layout.  Q output heads are permuted: head h -> col
                    # (h%4)*128 + (h//4)*64, so that after transpose head h
                    # sits at QT tile h%4, partition half (h//4)*64 == its
                    # kv head's partition base (wo rows are permuted on the
                    # host to match).
                    for (src, n_h, off) in ((ps_q, NQH, 0), (ps_kv, 2, QD)):
                        if n_h == NQH:
                            v3 = src[:, 0:512].rearrange(
                                "p (half blk d) -> p half blk d", half=2,
                                d=64)
                            o3 = qk[:, 0:512].rearrange(
                                "p (blk half d) -> p half blk d", half=2,
                                d=64)
                            sh = [128, 2, 4, 32]
                            c1 = cos_t[:, None, None, 0:32].broadcast_to(sh)
                            s1 = sin_t[:, None, None, 0:32].broadcast_to(sh)
                            c2 = cos_t[:, None, None, 32:64].broadcast_to(sh)
                            s2 = sin_t[:, None, None, 32:64].broadcast_to(sh)
                            q1, q2 = v3[:, :, :, 0:32], v3[:, :, :, 32:64]
                            oa, ob = o3[:, :, :, 0:32], o3[:, :, :, 32:64]
                        else:
                            v3 = src[:, 0:n_h * 64].rearrange(
                                "p (h d) -> p h d", d=64)
                            o3 = qk[:, off:off + n_h * 64].rearrange(
                                "p (h d) -> p h d", d=64)
                            sh = [128, n_h, 32]
                            c1 = cos_t[:, None, 0:32].broadcast_to(sh)
                            s1 = sin_t[:, None, 0:32].broadcast_to(sh)
                            c2 = cos_t[:, None, 32:64].broadcast_to(sh)
                            s2 = sin_t[:, None, 32:64].broadcast_to(sh)
                            q1, q2 = v3[:, :, 0:32], v3[:, :, 32:64]
                            oa, ob = o3[:, :, 0:32], o3[:, :, 32:64]
                        t1 = tmps.tile(sh, f32, tag="t1")
                        t2 = tmps.tile(sh, f32, tag="t2")
                        nc.vector.tensor_tensor(t1[:], q1, c1, OP.mult)
                        nc.vector.tensor_tensor(t2[:], q2, s1, OP.mult)
                        nc.vector.tensor_tensor(oa, t1[:], t2[:], OP.subtract)
                        nc.vector.tensor_tensor(t1[:], q2, c2, OP.mult)
                        nc.vector.tensor_tensor(t2[:], q1, s2, OP.mult)
                        nc.vector.tensor_tensor(ob, t1[:], t2[:], OP.add)
                    # ---- V evacuation (+ ones col already memset)
                    nc.vector.tensor_copy(
                        V[:, t, :, 0:64],
                        ps_kv[:, KD:2 * KD].rearrange(
                            "p (h d) -> p h d", d=64))
                    # ---- transpose q/k blocks into QT/KT
                    for db in range(5):
                        tp = pstp.tile([128, 128], bf16, tag="tp")
                        nc.tensor.transpose(
                            tp[:], qk[:, db * 128:(db + 1) * 128], ident[:])
                        if db < 4:
                            dst = QT[:, db, t * 128:(t + 1) * 128]
                        else:
                            dst = KT[:, t * 128:(t + 1) * 128]
                        nc.vector.tensor_copy(dst, tp[:])

            # ---------------- Phase C: attention (+ Wo interleaved) -----
            # Wo micro-op generator: yields per-call one PE matmul (or
            # schedules the DVE copy / out DMA when a group completes).
            with (
                tc.tile_pool(name="psS", bufs=2, space="PSUM") as psS,
                tc.tile_pool(name="psO", bufs=1, space="PSUM") as psO,
                tc.tile_pool(name="psW", bufs=2, space="PSUM") as psW,
            ):
                wo_state = {"t": 0, "nch": 0, "db": 0, "w_ps": None}

                def wo_step(max_t):
                    # emit one Wo matmul if work for token tiles < max_t
                    # remains; returns True if work was emitted
                    st = wo_state
                    if st["t"] >= max_t:
                        return False
                    if st["w_ps"] is None:
                        st["w_ps"] = psW.tile([128, 512], f32, tag="w",
                                              name="w_ps")
                    t, nch, db = st["t"], st["nch"], st["db"]
                    nc.tensor.matmul(
                        st["w_ps"][:],
                        lhsT=attnT[:, db, t * 128:(t + 1) * 128],
                        rhs=wo_sb[:, db, nch * 512:(nch + 1) * 512],
                        start=(db == 0), stop=(db == 3))
                    st["db"] += 1
                    if st["db"] == 4:
                        st["db"] = 0
                        o_c = outp.tile([128, 512], f32, tag="out",
                                        name="o_c")
                        nc.vector.tensor_copy(o_c[:], st["w_ps"][:])
                        nc.sync.dma_start(
                            out=out[t * 128:(t + 1) * 128,
                                    nch * 512:(nch + 1) * 512],
                            in_=o_c[:])
                        st["w_ps"] = None
                        st["nch"] += 1
                        if st["nch"] == 4:
                            st["nch"] = 0
                            st["t"] += 1
                    return True

                for qc in range(4):
                    for m in range(4):
                        # heads: kv0 head m (rows 0:64), kv1 head m
                        # (rows 64:128)
                        o_A = psO.tile([65, 512], f32, tag="oA")
                        o_B = psO.tile([65, 512], f32, tag="oB")
                        prev_p = None
                        for kt in range(NT):
                            sc = psS.tile([128, 1024], f32, tag="sc")
                            nc.tensor.matmul(
                                sc[:, 0:512],
                                lhsT=KT[0:64, kt * 128:(kt + 1) * 128],
                                rhs=QT[0:64, m, qc * 512:(qc + 1) * 512],
                                start=True, stop=True)
                            nc.tensor.matmul(
                                sc[:, 512:1024],
                                lhsT=KT[64:128, kt * 128:(kt + 1) * 128],
                                rhs=QT[64:128, m, qc * 512:(qc + 1) * 512],
                                start=True, stop=True)
                            p = pbf.tile([128, 1024], bf16, tag="p")
                            nc.scalar.activation(p[:], sc[:], AF.Exp,
                                                 scale=0.125)
                            if dbg and qc == 0 and m == 0 and kt == 0:
                                dsc = stage.tile([128, 2, 512], f32,
                                                 tag="dsc", bufs=1)
                                nc.vector.tensor_copy(
                                    dsc[:], sc.rearrange("p (a b) -> p a b",
                                                         a=2))
                                nc.gpsimd.dma_start(out=d_sc, in_=dsc[:])
                                dp = stage.tile([128, 2, 512], f32,
                                                tag="dp", bufs=1)
                                nc.vector.tensor_copy(
                                    dp[:], p.rearrange("p (a b) -> p a b",
                                                       a=2))
                                nc.gpsimd.dma_start(out=d_p, in_=dp[:])
                            # PV for the previous slot's p (keeps PE off
                            # the ACT critical path)
                            if prev_p is not None:
                                pp, pkt = prev_p
                                nc.tensor.matmul(
                                    o_A[:], lhsT=V[:, pkt, 0, :],
                                    rhs=pp[:, 0:512],
                                    start=(pkt == 0), stop=(pkt == NT - 1))
                                nc.tensor.matmul(
                                    o_B[:], lhsT=V[:, pkt, 1, :],
                                    rhs=pp[:, 512:1024],
                                    start=(pkt == 0), stop=(pkt == NT - 1))
                            if not wo_step(qc * 4) and qc == 0:
                                # no Wo work yet: full-width dummy matmul
                                # keeps the PE activity monitor above the
                                # un-throttle threshold during qc 0
                                dmy = psW.tile([128, 512], f32, tag="w",
                                               name="dmy")
                                nc.tensor.matmul(
                                    dmy[:], lhsT=wo_sb[:, 0, 0:128],
                                    rhs=wo_sb[:, 1, 0:512],
                                    start=True, stop=True)
                            prev_p = (p, kt)
                        pp, pkt = prev_p
                        nc.tensor.matmul(
                            o_A[:], lhsT=V[:, pkt, 0, :], rhs=pp[:, 0:512],
                            start=(pkt == 0), stop=(pkt == NT - 1))
                        nc.tensor.matmul(
                            o_B[:], lhsT=V[:, pkt, 1, :], rhs=pp[:, 512:1024],
                            start=(pkt == 0), stop=(pkt == NT - 1))
                        if dbg and qc == 0 and m == 0:
                            do_ = stage.tile([128, 2, 512], f32,
                                             tag="do", bufs=1)
                            nc.vector.tensor_copy(do_[0:65, 0, :], o_A[:])
                            nc.vector.tensor_copy(do_[0:65, 1, :], o_B[:])
                            nc.gpsimd.dma_start(out=d_o, in_=do_[:])
                        # normalize: row 64 carries the softmax denominator
                        for (o_ps, qr) in ((o_A, 0), (o_B, 64)):
                            rsum = rbp.tile([1, 512], f32, tag="rsum",
                                            bufs=1)
                            nc.vector.tensor_copy(rsum[:], o_ps[64:65, :])
                            recip = rbp.tile([1, 512], f32, tag="recip",
                                             bufs=1)
                            nc.vector.reciprocal_approx_fast(recip[:],
                                                             rsum[:])
                            rb = rbp.tile([64, 512], f32, tag="rb", bufs=1)
                            nc.gpsimd.partition_broadcast(rb[:], recip[:])
                            nc.vector.tensor_tensor(
                                attnT[qr:qr + 64, m,
                                      qc * 512:(qc + 1) * 512],
                                o_ps[0:64, :], rb[:], OP.mult)

                # ---------------- Phase D: drain remaining Wo ----------
                while wo_state["t"] < NT:
                    wo_step(NT)

                if dbg:
                    for r in range(NR):
                        nc.gpsimd.dma_start(out=d_xt[:, r, :], in_=xT[r][:])
                    for (dt_, st_) in ((d_sin, sin_all), (d_cos, cos_all),
                                       (d_qt, QT), (d_kt, KT), (d_v, V),
                                       (d_at, attnT)):
                        nc.gpsimd.dma_start(out=dt_, in_=st_[:])

    nc.compile()
    return nc


def _get_nc():
    if "nc" not in _CACHE:
        _CACHE["nc"] = _build()
    return _CACHE["nc"]


def _shard(inputs):
    hs = np.ascontiguousarray(np.asarray(inputs["hidden_states"], np.float32))
    Wq = np.asarray(inputs["Wq"], np.float32)
    Wk = np.asarray(inputs["Wk"], np.float32)
    Wv = np.asarray(inputs["Wv"], np.float32)
    Wo = np.asarray(inputs["Wo"], np.float32)
    in_maps = []
    for i in range(8):
        b, g = divmod(i, 4)
        in_maps.append({
            "x": hs[b],
            "wq": np.ascontiguousarray(Wq[:, g * 512:(g + 1) * 512]),
            "wk": np.ascontiguousarray(Wk[:, g * 128:(g + 1) * 128]),
            "wv": np.ascontiguousarray(Wv[:, g * 128:(g + 1) * 128]),
            "wo": np.ascontiguousarray(
                Wo[g * 512:(g + 1) * 512, :].reshape(8, 64, HID)[
                    [0, 4, 1, 5, 2, 6, 3, 7]].reshape(512, HID)),
        })
    return in_maps


def run(inputs, trace=False, tmpdir=None):
    """Run on 8 cores; returns (output [2,2048,2048] f32, exec_time_ns)."""
    from concourse.bass_utils import run_bass_kernel_spmd

    nc = _get_nc()
    in_maps = _shard(inputs)
    kwargs = {}
    if trace:
        import sys, types
        from trn_agent_boot.trn_boot import _ntff_profile_via_ctypes
        if "antenv.axon_hooks" not in sys.modules:
            mod = types.ModuleType("antenv.axon_hooks")
            hook = _ntff_profile_via_ctypes("/opt/axon/libaxon_pjrt.so")
            mod.get_axon_ntff_profile_hook = lambda: hook
            sys.modules["antenv.axon_hooks"] = mod
        import concourse.bass_utils as bu
        bu.upload_artifacts = lambda d: f"local://{d}"
        kwargs = {"trace": True, "tmpdir": tmpdir}
    res = run_bass_kernel_spmd(nc, in_maps, core_ids=list(range(8)), **kwargs)
    full = np.zeros((2, S, HID), np.float32)
    for i in range(8):
        b = i // 4
        full[b] += res.results[i]["out"]
    return full, res.exec_time_ns


def kernel(**inputs):
    out, _ = run(inputs)
    return out
